# revision 23
# baseline (speedup 1.0000x reference)
"""Trainium2 Bass kernel for nn_Adapter (per-token candidate attention + MLP + LN).

Data-parallel over tokens across 8 NeuronCores. v2: fp8-e4m3 DoubleRow matmuls
with software-pipelined emission.

Key design points vs the bf16 baseline:
  - All five projections (q/k/v/MLP1/MLP2) run in fp8e4 with
    perf_mode=DoubleRow: contraction of 256 rows per PE instruction
    (2x MAC throughput). Weights are scaled x64 (x128 for Wc) on the host so
    their values sit in e4m3's normal range; the scale is folded back into the
    softmax exponent scale, the attention-normalization reciprocal, and the
    gelu/copy activation scales.
  - x and c are pre-transposed to feature-major fp8 on the HOST, so the 54
    PE transposes per 128-token tile (and their PSUM->SBUF copies) vanish.
    c DMA shrinks 4x (fp8 vs f32).
  - Attention math (q*k scores, softmax, e-weighted v combine) in bf16 on DVE.
  - PE emission is software-pipelined: each chunk's MLP is interleaved with the
    NEXT chunk's qkv tiles so the PE never waits on the DVE attention chain;
    ctx transposes are batched at chunk level; k|v share one 1536-wide PSUM
    tile (bufs=2) giving ACT a two-candidate drain window.
  - LayerNorm fused on DVE/ACT as before; residual uses bf16 x.
"""

import numpy as np
import ml_dtypes

import concourse.bass as bass
import concourse.mybir as mybir
import concourse.tile as tile
from concourse.bass_utils import run_bass_kernel_spmd

F32 = mybir.dt.float32
BF16 = mybir.dt.bfloat16
FP8 = mybir.dt.float8e4
AX = mybir.AxisListType.X
ALU = mybir.AluOpType
ACTF = mybir.ActivationFunctionType
DR = mybir.MatmulPerfMode.DoubleRow

HID = 768
NH = 12
HD = 64
NCAND = 8
NCORES = 8
EPS = 1e-12
NI = HID // 128          # 6 input-feature chunks
NJ4 = 4 * HID // 128     # 24 hidden chunks
SW = 64.0                # host-side scale on Wq/Wk/Wv/Wt
SWC = 128.0              # host-side scale on Wc
ESCALE = (1.0 / np.sqrt(HD)) / (SW * SW)

_CACHE = {}


def _split_excess_waits(nc, max_waits=1):
    """walrus in this container only packs ONE sync-wait per ISA instruction;
    move excess on_wait entries onto inserted same-engine Drain carriers."""
    for func in nc.m.functions:
        for block in func.blocks:
            new = []
            for inst in block.instructions:
                si = inst.sync_info
                if si is not None:
                    waits = list(si.on_wait)
                    if len(waits) > max_waits:
                        k = 0
                        while len(waits) > max_waits:
                            chunk, waits = waits[:max_waits], waits[max_waits:]
                            carrier = mybir.InstNoOp(
                                name=f"{inst.name}-ws{k}", engine=inst.engine,
                                sync_info=mybir.SyncInfo(on_wait=chunk,
                                                         on_update=[]))
                            nc.register_instruction(carrier, overwrite=True)
                            new.append(carrier)
                            k += 1
                        inst.sync_info = mybir.SyncInfo(
                            on_wait=waits, on_update=list(si.on_update))
                new.append(inst)
            block.instructions = new


def build(tc_tokens, has_b, has_aff):
    """Build the single-core Bass graph (same graph runs SPMD on all cores)."""
    nt = tc_tokens // 128
    nc = bass.Bass()

    id_d = nc.dram_tensor("idb", [128, 128], BF16, kind="ExternalInput")
    idf_d = nc.dram_tensor("idf", [128, 128], F32, kind="ExternalInput")
    xT_d = nc.dram_tensor("xT", [nt, 128, NI, 128], FP8, kind="ExternalInput")
    xb_d = nc.dram_tensor("xb", [tc_tokens, HID], BF16, kind="ExternalInput")
    cT_d = nc.dram_tensor("cT", [nt, 128, NCAND, NI, 128], FP8,
                          kind="ExternalInput")
    m_d = nc.dram_tensor("m", [128, tc_tokens // 128, NCAND], F32,
                         kind="ExternalInput")
    wq_d = nc.dram_tensor("wq", [128, NI, HID], FP8, kind="ExternalInput")
    wk_d = nc.dram_tensor("wk", [128, NI, HID], FP8, kind="ExternalInput")
    wv_d = nc.dram_tensor("wv", [128, NI, HID], FP8, kind="ExternalInput")
    wt_d = nc.dram_tensor("wt", [128, NJ4, NI, 128], FP8, kind="ExternalInput")
    wc_d = nc.dram_tensor("wc", [128, NI, NJ4, 128], FP8, kind="ExternalInput")
    if has_b:
        bq_d = nc.dram_tensor("bq", [HID], F32, kind="ExternalInput")   # x64
        bk_d = nc.dram_tensor("bk", [HID], F32, kind="ExternalInput")   # x64
        bv_d = nc.dram_tensor("bv", [HID], F32, kind="ExternalInput")
        bt_d = nc.dram_tensor("bt", [4 * HID], F32, kind="ExternalInput")
        bc_d = nc.dram_tensor("bc", [HID], F32, kind="ExternalInput")
    if has_aff:
        ga_d = nc.dram_tensor("ga", [HID], F32, kind="ExternalInput")
        be_d = nc.dram_tensor("be", [HID], F32, kind="ExternalInput")
    o_d = nc.dram_tensor("out", [tc_tokens, HID], F32, kind="ExternalOutput")

    with tile.TileContext(nc) as tc:
        consts = tc.alloc_tile_pool(name="consts", bufs=1)
        wpool = tc.alloc_tile_pool(name="wpool", bufs=1)
        xp = tc.alloc_tile_pool(name="xp", bufs=2)
        ctp = tc.alloc_tile_pool(name="ctp", bufs=2)
        qp = tc.alloc_tile_pool(name="qp", bufs=2)
        kp = tc.alloc_tile_pool(name="kp", bufs=3)
        vp = tc.alloc_tile_pool(name="vp", bufs=2)
        pp = tc.alloc_tile_pool(name="pp", bufs=3)
        sm = tc.alloc_tile_pool(name="sm", bufs=4)
        cxp = tc.alloc_tile_pool(name="cxp", bufs=2)
        cbp = tc.alloc_tile_pool(name="cbp", bufs=6)
        chk = tc.alloc_tile_pool(name="chk", bufs=2)
        h1p = tc.alloc_tile_pool(name="h1p", bufs=1)
        lnp = tc.alloc_tile_pool(name="lnp", bufs=2)

        # separate bank-aligned k/v PSUM tiles (2 banks each after rounding):
        # ps_k 2x2 + ps_v 1x2 + ps_mlp 2x1 = 8 banks exactly.
        ps_k = tc.alloc_tile_pool(name="ps_k", bufs=2, space="PSUM")
        ps_v = tc.alloc_tile_pool(name="ps_v", bufs=1, space="PSUM")
        ps_mlp = tc.alloc_tile_pool(name="ps_mlp", bufs=2, space="PSUM")

        ident_b = consts.tile([128, 128], BF16)
        nc.sync.dma_start(out=ident_b, in_=id_d[:, :])
        ident_f = consts.tile([128, 128], F32)
        nc.sync.dma_start(out=ident_f, in_=idf_d[:, :])
        m_all = consts.tile([128, nt, NCAND], F32)
        nc.sync.dma_start(out=m_all, in_=m_d[:, :, :])
        # mask-only derived tensors, computed once for ALL tiles:
        # omesc = ESCALE*(1-m) (per-candidate exp scale, 0 for masked),
        # notall = 1 unless every candidate is masked.
        omesc_all = consts.tile([128, nt, NCAND], F32)
        nc.vector.tensor_scalar(out=omesc_all, in0=m_all, scalar1=-ESCALE,
                                scalar2=ESCALE, op0=ALU.mult, op1=ALU.add)
        msum_all = consts.tile([128, nt], F32)
        nc.vector.tensor_reduce(out=msum_all, in_=m_all, axis=AX, op=ALU.add)
        notall_all = consts.tile([128, nt], F32)
        nc.vector.tensor_scalar(out=notall_all, in0=msum_all,
                                scalar1=float(NCAND) - 0.5,
                                scalar2=None, op0=ALU.is_lt)
        ceps = consts.tile([128, 1], F32)
        nc.vector.memset(ceps, EPS)

        # resident fp8 weights (host pre-transposed + pre-scaled).
        # qkv weights go on the sync queue (needed immediately); the big MLP
        # weights ride the gpsimd SWDGE queue so tile-0's cT isn't stuck
        # behind 4.5 MB on the sync queue.
        wq_sb = wpool.tile([128, NI, HID], FP8)
        nc.sync.dma_start(out=wq_sb, in_=wq_d[:, :, :])
        wk_sb = wpool.tile([128, NI, HID], FP8)
        nc.sync.dma_start(out=wk_sb, in_=wk_d[:, :, :])
        wv_sb = wpool.tile([128, NI, HID], FP8)
        nc.sync.dma_start(out=wv_sb, in_=wv_d[:, :, :])
        wt_sb = wpool.tile([128, NJ4, NI, 128], FP8)
        nc.gpsimd.dma_start(out=wt_sb, in_=wt_d[:, :, :, :])
        wc_sb = wpool.tile([128, NI, NJ4, 128], FP8)
        nc.gpsimd.dma_start(out=wc_sb, in_=wc_d[:, :, :, :])

        if has_b:
            bq_rep = consts.tile([128, HID], F32)
            nc.gpsimd.dma_start(out=bq_rep, in_=bq_d.to_broadcast([128, HID]))
            bk_rep = consts.tile([128, HID], F32)
            nc.gpsimd.dma_start(out=bk_rep, in_=bk_d.to_broadcast([128, HID]))
            bv_rep = consts.tile([128, HID], F32)
            nc.gpsimd.dma_start(out=bv_rep, in_=bv_d.to_broadcast([128, HID]))
            bt_sb = consts.tile([128, NJ4], F32)
            nc.sync.dma_start(out=bt_sb, in_=bt_d.rearrange("(c p) -> p c", p=128))
            bc_sb = consts.tile([128, NI], F32)
            nc.sync.dma_start(out=bc_sb, in_=bc_d.rearrange("(c p) -> p c", p=128))
        if has_aff:
            ga_rep = consts.tile([128, HID], F32)
            nc.gpsimd.dma_start(out=ga_rep, in_=ga_d.to_broadcast([128, HID]))
            be_rep = consts.tile([128, HID], F32)
            nc.gpsimd.dma_start(out=be_rep, in_=be_d.to_broadcast([128, HID]))

        ctx_bs = {}      # tt -> ctx_b tile
        ctxTs = {}       # chunk-id -> ctxT tile
        h1Ts = {}
        o2Ts = {}

        def do_tile(tt, fill=None):
            xT = xp.tile([128, NI, 128], FP8, tag="xT")
            nc.sync.dma_start(out=xT, in_=xT_d[tt])
            cT = ctp.tile([128, NCAND, NI, 128], FP8, tag="cT")
            nc.sync.dma_start(out=cT, in_=cT_d[tt])
            m_t = m_all[:, tt, :]

            # ---- q projection (DoubleRow fp8) ----
            q_ps = ps_k.tile([128, HID], F32, tag="a")
            for ip in range(3):
                st, sp = (ip == 0), (ip == 2)
                nc.tensor.matmul(q_ps[:, :512], xT[:, 2 * ip:2 * ip + 2, :],
                                 wq_sb[:, 2 * ip:2 * ip + 2, :512],
                                 start=st, stop=sp, perf_mode=DR)
                nc.tensor.matmul(q_ps[:, 512:], xT[:, 2 * ip:2 * ip + 2, :],
                                 wq_sb[:, 2 * ip:2 * ip + 2, 512:],
                                 start=st, stop=sp, perf_mode=DR)
            q_sb = qp.tile([128, HID], BF16, tag="q_sb")
            if has_b:
                q_f = qp.tile([128, HID], F32, tag="q_f")
                nc.scalar.copy(q_f, q_ps)
                nc.vector.tensor_add(q_sb, q_f, bq_rep)
            else:
                nc.scalar.copy(q_sb, q_ps)

            # ---- per-candidate k+v projections, scores, exp and ctx ----
            # softmax normalization is factored out: ctx_unnorm accumulates
            # exp(score)-weighted v per candidate (overlapped with the next
            # candidate's matmuls); one final multiply applies 1/sum(exp).
            # Masked scores are zeroed before the exp (exp(0)=1 matches the
            # reference's 1e-10 fill). prod+reduce run pair-batched on DVE to
            # halve the per-op pipeline-drain cost; the e-weighted v multiply
            # runs on GPSIMD, the accumulate add on DVE.
            omesc = omesc_all[:, tt, :]
            notall = notall_all[:, tt:tt + 1]
            if has_b:
                prodf = pp.tile([128, HID], F32, tag="prodf")
                nc.vector.tensor_mul(prodf, q_sb, bk_rep)
                qbk = sm.tile([128, NH, 1], F32, tag="qbk")
                nc.vector.tensor_reduce(
                    out=qbk, in_=prodf.rearrange("p (h d) -> p h d", h=NH),
                    axis=AX, op=ALU.add)
            scores = sm.tile([128, NH, NCAND], F32, tag="scores")
            e8 = sm.tile([128, NH, NCAND], BF16, tag="e8")
            v_all = vp.tile([128, NCAND, HID], BF16, tag="v_all")
            ctx = cxp.tile([128, NH, HD], BF16, tag="ctx")

            def emit_exp(j):
                nc.scalar.activation(e8[:, :, j:j + 1], scores[:, :, j:j + 1],
                                     ACTF.Exp, scale=omesc[:, j:j + 1])

            def emit_ctx(j):
                # broadcast-mult on GPSIMD (DVE would drop to 1x on the
                # stride-0 e view anyway); cheap dense bf16 add on DVE at 2x.
                v3 = v_all[:, j, :].rearrange("p (h d) -> p h d", h=NH)
                e_b = e8[:, :, j:j + 1].broadcast_to([128, NH, HD])
                if j == 0:
                    nc.gpsimd.tensor_mul(ctx, v3, e_b)
                else:
                    prodv = cxp.tile([128, NH, HD], BF16, tag="prodv")
                    nc.gpsimd.tensor_mul(prodv, v3, e_b)
                    nc.vector.tensor_add(ctx, ctx, prodv)

            for n in range(NCAND):
                # v MMs lead each candidate, so v_ps gets the double-buffered
                # pool (2-candidate ACT drain window); k MMs trail by ~1.1us
                # and tolerate the single-buffered pool.
                v_ps = ps_k.tile([128, HID], F32, tag="a")
                k_ps = ps_v.tile([128, HID], F32, tag="v")
                for ip in range(3):
                    st, sp = (ip == 0), (ip == 2)
                    pair = cT[:, n, 2 * ip:2 * ip + 2, :]
                    nc.tensor.matmul(v_ps[:, :512], pair,
                                     wv_sb[:, 2 * ip:2 * ip + 2, :512],
                                     start=st, stop=sp, perf_mode=DR)
                    nc.tensor.matmul(v_ps[:, 512:], pair,
                                     wv_sb[:, 2 * ip:2 * ip + 2, 512:],
                                     start=st, stop=sp, perf_mode=DR)
                for ip in range(3):
                    st, sp = (ip == 0), (ip == 2)
                    pair = cT[:, n, 2 * ip:2 * ip + 2, :]
                    nc.tensor.matmul(k_ps[:, :512], pair,
                                     wk_sb[:, 2 * ip:2 * ip + 2, :512],
                                     start=st, stop=sp, perf_mode=DR)
                    nc.tensor.matmul(k_ps[:, 512:], pair,
                                     wk_sb[:, 2 * ip:2 * ip + 2, 512:],
                                     start=st, stop=sp, perf_mode=DR)
                nc.scalar.copy(v_all[:, n, :], v_ps)
                k_sb = kp.tile([128, HID], BF16, tag="k_sb")
                nc.scalar.copy(k_sb, k_ps)
                # lag-1 exp/mult/add: emitted one candidate late so neither
                # the ACT nor the DVE queue ever waits head-of-line on the
                # other engine's freshest result.
                if n >= 1:
                    emit_exp(n - 1)
                prod = pp.tile([128, HID], BF16, tag="prod")
                nc.vector.tensor_mul(prod, k_sb, q_sb)
                sc_n = scores[:, :, n:n + 1]
                nc.vector.tensor_reduce(
                    out=sc_n,
                    in_=prod.rearrange("p (h d) -> p h d", h=NH),
                    axis=AX, op=ALU.add)
                if has_b:
                    nc.vector.tensor_add(sc_n, sc_n, qbk)
                if n >= 1:
                    emit_ctx(n - 1)
                for th in (fill or {}).get(n, []):
                    th()
            emit_exp(NCAND - 1)
            emit_ctx(NCAND - 1)

            # ---- softmax normalization (factored) ----
            esum = sm.tile([128, NH], F32, tag="esum")
            nc.vector.tensor_reduce(out=esum, in_=e8, axis=AX, op=ALU.add)
            recip = sm.tile([128, NH], F32, tag="recip")
            nc.vector.reciprocal(recip, esum)
            # fold all-masked zeroing AND the 1/SW v-descale into recip
            nc.vector.tensor_scalar(out=recip, in0=recip, scalar1=notall,
                                    scalar2=1.0 / SW, op0=ALU.mult,
                                    op1=ALU.mult)
            ctx_b = cbp.tile([128, HID], BF16, tag="ctx_b")
            ctx_b3 = ctx_b.rearrange("p (h d) -> p h d", h=NH)
            r_b = recip.unsqueeze(2).broadcast_to([128, NH, HD])
            if has_b:
                ctxn = cxp.tile([128, NH, HD], BF16, tag="ctxn")
                nc.vector.tensor_mul(ctxn, ctx, r_b)
                nc.vector.scalar_tensor_tensor(
                    out=ctx_b3,
                    in0=bv_rep.rearrange("p (h d) -> p h d", h=NH),
                    scalar=notall, in1=ctxn, op0=ALU.mult, op1=ALU.add)
            else:
                nc.vector.tensor_mul(ctx_b3, ctx, r_b)
            ctx_bs[tt] = ctx_b

        def ctr_piece(ci, tloc, tt):
            """transpose one tile's ctx into the chunk's K-major fp8 buffer"""
            if ci not in ctxTs:
                ctxT_new = chk.tile([128, NI, 512], FP8, tag="ctxT")
                ctxTs[ci] = ctxT_new
            ctxT = ctxTs[ci]
            ctx_b = ctx_bs.pop(tt)
            ctr_ps = ps_k.tile([128, HID], F32, tag="a")
            for i in range(NI):
                nc.tensor.matmul(ctr_ps[:, i * 128:(i + 1) * 128],
                                 ctx_b[:, i * 128:(i + 1) * 128], ident_b,
                                 start=True, stop=True)
            nc.scalar.copy(
                ctxT[:, :, tloc * 128:(tloc + 1) * 128],
                ctr_ps.rearrange("p (c j) -> p c j", c=NI))

        def mlp1_piece(ci, j, cw):
            if ci not in h1Ts:
                h1T_new = h1p.tile([128, NJ4, 512], FP8, tag="h1T")
                h1Ts[ci] = h1T_new
            h1T = h1Ts[ci]
            ctxT = ctxTs[ci]
            h1_ps = ps_mlp.tile([128, 512], F32, tag="mlpps")
            for ip in range(3):
                nc.tensor.matmul(h1_ps[:, :cw],
                                 wt_sb[:, j, 2 * ip:2 * ip + 2, :],
                                 ctxT[:, 2 * ip:2 * ip + 2, :cw],
                                 start=(ip == 0), stop=(ip == 2),
                                 perf_mode=DR)
            nc.scalar.activation(h1T[:, j, :cw], h1_ps[:, :cw], ACTF.Gelu,
                                 scale=1.0 / SW,
                                 bias=(bt_sb[:, j:j + 1] if has_b else 0.0))

        def mlp2_piece(ci, o, cw):
            if ci not in o2Ts:
                o2T_new = chk.tile([128, NI, 512], F32, tag="o2T")
                o2Ts[ci] = o2T_new
            o2T = o2Ts[ci]
            h1T = h1Ts[ci]
            o2_ps = ps_mlp.tile([128, 512], F32, tag="mlpps")
            for jp in range(NJ4 // 2):
                nc.tensor.matmul(o2_ps[:, :cw],
                                 wc_sb[:, o, 2 * jp:2 * jp + 2, :],
                                 h1T[:, 2 * jp:2 * jp + 2, :cw],
                                 start=(jp == 0), stop=(jp == NJ4 // 2 - 1),
                                 perf_mode=DR)
            nc.scalar.activation(o2T[:, o, :cw], o2_ps[:, :cw], ACTF.Copy,
                                 scale=1.0 / SWC,
                                 bias=(bc_sb[:, o:o + 1] if has_b else 0.0))

        def ln_piece(ci, tloc, tt):
            o2T = o2Ts[ci]
            if True:
                t0 = tt * 128
                o2n_ps = ps_k.tile([128, HID], F32, tag="a")
                for o in range(NI):
                    nc.tensor.transpose(o2n_ps[:, o * 128:(o + 1) * 128],
                                        o2T[:, o, tloc * 128:(tloc + 1) * 128],
                                        ident_f)
                x_f = lnp.tile([128, HID], BF16, tag="x_f")
                nc.sync.dma_start(out=x_f, in_=xb_d[t0:t0 + 128, :])

                y_sb = lnp.tile([128, HID], F32, tag="y_sb")
                sums = sm.tile([128, 1], F32, tag="sums")
                nc.vector.scalar_tensor_tensor(
                    out=y_sb, in0=o2n_ps, scalar=1.0, in1=x_f,
                    op0=ALU.mult, op1=ALU.add, accum_out=sums)
                out_sb = lnp.tile([128, HID], F32, tag="out_sb")
                sumsq = sm.tile([128, 1], F32, tag="sumsq")
                nc.vector.scalar_tensor_tensor(
                    out=out_sb, in0=y_sb, scalar=1.0, in1=y_sb,
                    op0=ALU.mult, op1=ALU.mult, accum_out=sumsq)
                mean = sm.tile([128, 1], F32, tag="mean")
                nc.vector.tensor_scalar(out=mean, in0=sums, scalar1=1.0 / HID,
                                        scalar2=None, op0=ALU.mult)
                msq = sm.tile([128, 1], F32, tag="msq")
                nc.vector.tensor_mul(msq, mean, mean)
                var = sm.tile([128, 1], F32, tag="var")
                nc.vector.tensor_scalar(out=var, in0=sumsq, scalar1=1.0 / HID,
                                        scalar2=msq, op0=ALU.mult,
                                        op1=ALU.subtract)
                # rstd = exp(-0.5 * ln(var + eps)) — Ln/Exp share ACT tables
                lnv = sm.tile([128, 1], F32, tag="lnv")
                nc.scalar.activation(lnv, var, ACTF.Ln, bias=ceps)
                rstd = sm.tile([128, 1], F32, tag="rstd")
                nc.scalar.activation(rstd, lnv, ACTF.Exp, scale=-0.5)

                nc.vector.tensor_scalar(out=out_sb, in0=y_sb, scalar1=mean,
                                        scalar2=rstd, op0=ALU.subtract,
                                        op1=ALU.mult)
                if has_aff:
                    nc.vector.tensor_mul(out_sb, out_sb, ga_rep)
                    nc.vector.tensor_add(out_sb, out_sb, be_rep)
                nc.sync.dma_start(out=o_d[t0:t0 + 128, :], in_=out_sb)

        chunks = [list(range(s, min(s + 4, nt))) for s in range(0, nt, 4)]
        if nt == 8:
            # fine-grained weave: chunk-0's MLP/transpose/LN work is emitted
            # in small pieces BETWEEN candidates of tiles 4-7, so the PE
            # absorbs the (PE-dense) MLP while the DVE-bound candidate loops
            # run; only chunk-1's MLP remains as a PE-dense tail.
            for tt in (0, 1, 2, 3):
                do_tile(tt)
            do_tile(4)
            for tloc, tt in enumerate(chunks[0]):
                ctr_piece(0, tloc, tt)
            for j in range(NJ4):
                mlp1_piece(0, j, 512)
            do_tile(5)
            for o in range(NI):
                mlp2_piece(0, o, 512)
            for tloc, tt in enumerate(chunks[0]):
                ln_piece(0, tloc, tt)
            do_tile(6)
            do_tile(7)
            for tloc, tt in enumerate(chunks[1]):
                ctr_piece(1, tloc, tt)
            for j in range(NJ4):
                mlp1_piece(1, j, 512)
            for o in range(NI):
                mlp2_piece(1, o, 512)
            for tloc, tt in enumerate(chunks[1]):
                ln_piece(1, tloc, tt)
        else:
            for ci, chunk in enumerate(chunks):
                cw = 128 * len(chunk)
                for tt in chunk:
                    do_tile(tt)
                for tloc, tt in enumerate(chunk):
                    ctr_piece(ci, tloc, tt)
                for j in range(NJ4):
                    mlp1_piece(ci, j, cw)
                for o in range(NI):
                    mlp2_piece(ci, o, cw)
                for tloc, tt in enumerate(chunk):
                    ln_piece(ci, tloc, tt)

        for p in reversed((consts, wpool, xp, ctp, qp, kp, vp, pp, sm, cxp,
                           cbp, chk, h1p, lnp, ps_k, ps_v, ps_mlp)):
            p.release()
    _split_excess_waits(nc)
    return nc


def _prep(inputs):
    ins = {k: np.asarray(v) for k, v in inputs.items()}
    x = ins["layer_output"].astype(np.float32)
    c = ins["candidates_embeddings"].astype(np.float32)
    m = ins["candidates_mask"].astype(np.float32)
    B, S, H = x.shape
    T = B * S
    n_ = c.shape[2]
    assert H == HID and n_ == NCAND and T % (NCORES * 128) == 0

    has_b = any(np.any(ins[k] != 0) for k in ("bq", "bk", "bv", "bt", "bc"))
    has_aff = bool(np.any(ins["gamma"] != 1) or np.any(ins["beta"] != 0))

    bf = ml_dtypes.bfloat16
    e4 = ml_dtypes.float8_e4m3

    def wq_prep(w, s):
        # [out, in] -> [p, i, j] with value w[j, i*128+p] * s
        wt = np.ascontiguousarray(w.astype(np.float32).T * s)   # [in, out]
        return np.ascontiguousarray(
            wt.reshape(NI, 128, HID).transpose(1, 0, 2)).astype(e4)

    wt_f = ins["Wt"].astype(np.float32)      # [3072, 768]
    wc_f = ins["Wc"].astype(np.float32)      # [768, 3072]
    wt8 = np.ascontiguousarray(
        (wt_f.T * SW).reshape(NI, 128, NJ4, 128).transpose(1, 2, 0, 3)
    ).astype(e4)                              # [p, j, i, m]
    wc8 = np.ascontiguousarray(
        (wc_f.T * SWC).reshape(NJ4, 128, NI, 128).transpose(1, 2, 0, 3)
    ).astype(e4)                              # [p, o, j, m]

    weights = {
        "idb": np.eye(128, dtype=np.float32).astype(bf),
        "idf": np.eye(128, dtype=np.float32),
        "wq": wq_prep(ins["Wq"], SW),
        "wk": wq_prep(ins["Wk"], SW),
        "wv": wq_prep(ins["Wv"], SW),
        "wt": wt8,
        "wc": wc8,
    }
    if has_b:
        weights["bq"] = ins["bq"].astype(np.float32) * SW
        weights["bk"] = ins["bk"].astype(np.float32) * SW
        weights["bv"] = ins["bv"].astype(np.float32)
        weights["bt"] = ins["bt"].astype(np.float32)
        weights["bc"] = ins["bc"].astype(np.float32)
    if has_aff:
        weights["ga"] = ins["gamma"].astype(np.float32)
        weights["be"] = ins["beta"].astype(np.float32)

    tc_tokens = T // NCORES
    nt = tc_tokens // 128
    xf = x.reshape(T, H)
    cf = c.reshape(T, NCAND, H)
    mf = m.reshape(T, NCAND)
    in_maps = []
    for k in range(NCORES):
        sl = slice(k * tc_tokens, (k + 1) * tc_tokens)
        xs = xf[sl]
        cs = cf[sl]
        xT8 = np.ascontiguousarray(
            xs.reshape(nt, 128, NI, 128).transpose(0, 3, 2, 1)).astype(e4)
        cT8 = np.ascontiguousarray(
            cs.reshape(nt, 128, NCAND, NI, 128).transpose(0, 4, 2, 3, 1)
        ).astype(e4)
        m_r = np.ascontiguousarray(
            mf[sl].reshape(nt, 128, NCAND).transpose(1, 0, 2))
        im = {"xT": xT8,
              "xb": np.ascontiguousarray(xs).astype(bf),
              "cT": cT8,
              "m": m_r}
        im.update(weights)
        in_maps.append(im)
    return in_maps, tc_tokens, has_b, has_aff, (B, S, H)


def kernel(**inputs):
    in_maps, tc_tokens, has_b, has_aff, (B, S, H) = _prep(inputs)
    key = (tc_tokens, has_b, has_aff)
    if key not in _CACHE:
        _CACHE[key] = build(*key)
    nc = _CACHE[key]
    res = run_bass_kernel_spmd(nc, in_maps, core_ids=list(range(NCORES)))
    out = np.concatenate([res.results[i]["out"] for i in range(NCORES)], axis=0)
    return out.reshape(B, S, H).astype(np.float32)


# exposed for test.py profiling
def kernel_profiled(**inputs):
    in_maps, tc_tokens, has_b, has_aff, (B, S, H) = _prep(inputs)
    key = (tc_tokens, has_b, has_aff)
    if key not in _CACHE:
        _CACHE[key] = build(*key)
    nc = _CACHE[key]
    res = run_bass_kernel_spmd(nc, in_maps, core_ids=list(range(NCORES)),
                               trace=True)
    out = np.concatenate([res.results[i]["out"] for i in range(NCORES)], axis=0)
    return out.reshape(B, S, H).astype(np.float32), res
